# revision 43
# baseline (speedup 1.0000x reference)
"""Trainium2 Bass kernel for nn_CapsuleLayerTSV (capsule routing over 40 adapters).

Strategy (8 NeuronCores, two SPMD NEFFs, no collectives), all fp16 on the wire
(11-bit mantissa ~ f32r precision; routing softmax is too sensitive for bf16 —
measured 8e-2 rel err with bf16 priors vs 1e-3 with fp16):

  Phase 1 (expert-parallel): allowed adapters (tsv[t] != 0) sharded 3-per-core.
    Each core computes priors[b, n*d] = x[:, k, :] @ W[k] as fp16 matmuls
    (1 cyc/row) with f32 PSUM accumulate, chunk-outer so the PE chases the
    DMA stream. Outputs priors in fp16 (halves the store); a per-core f32
    adapter-sum partial (iteration-1 vote) rides on the DVE.
  Host: reassemble priors (fp16 -> f32), all-reduce the vote partials,
    re-shard by the output's flat row space (output row r uses flat vote
    elements 3r..3r+2; core c gets pairs s in [96c, 96c+96), s = n*256 + b).
  Phase 2 (pair-parallel): 3-iteration dynamic routing for 96 pairs per core,
    then u[6400,3] @ lwg[3,768] with the sigmoid gate folded in on host.
    - squash factored into a per-pair scalar: <P_k, squash(v)> =
      g * <P_k, v> with g = dinv*sqrt(sq)/(1+sq), so the squashed vote is
      never materialized and agreements run on the raw vote.
    - sqrt via exp(0.5*ln): keeps ACT on the single natural_log_exp table
      (square/ln/exp/copy) -- zero act-table reloads (4x 1.28us saved).
    - tsv values on allowed adapters are identically 1 (tril of ones), so
      all tsv multiplies drop out (asserted on host).
    - agreement + vote run on DVE (walrus rejects TensorScalarPtr on Pool,
      and Pool TT+reduce pairs are slower than DVE's fused op; ACT runs the
      square/ln/exp scalar chain concurrently).
    - fp16 output store (halves the 19.7MB/core store to 9.8MB ~ 28us at
      the ~355GB/s per-core roofline); host upcasts to f32.
    - vote deinterleave to u^T entirely in SBUF (no DRAM bounce).
"""

import sys

sys.path.insert(0, "/opt/trn_rl_repo")

import numpy as np

import concourse.bass as bass
import concourse.mybir as mybir
import concourse.tile as tile
from concourse.bass_utils import run_bass_kernel_spmd

F32 = mybir.dt.float32
F16 = mybir.dt.float16
AX = mybir.AxisListType
ALU = mybir.AluOpType
ACTF = mybir.ActivationFunctionType

NC = 8
B = 256
ADP = 40
CAPS = 3
INCH = 600
D = 200
M = 768
ND = CAPS * D  # 600
PP = CAPS * B // NC  # 96 (n,b) pairs per core in phase 2
ROWS_PER_CORE = PP * D // CAPS  # 6400 output rows per core
JCH = ROWS_PER_CORE // 128  # 50 j-chunks

_K_CHUNKS = [(0, 128), (128, 128), (256, 128), (384, 128), (512, 88)]

_BUILD_CACHE = {}


def _split_multiwait_waits(nc):
    """walrus caps sync-waits at ONE per instruction. For instructions executed
    by an in-order engine sequencer (everything except queue-executed DMAs),
    splitting the wait list across preceding 1-wait NoOps/Drains on the same
    engine is semantics-preserving."""
    for fn in nc.m.functions:
        for blk in fn.blocks:
            out = []
            for inst in blk.instructions:
                si = getattr(inst, "sync_info", None)
                if (
                    si is not None
                    and si.on_wait
                    and len(si.on_wait) > 1
                    and not isinstance(inst, mybir.InstDMACopy)
                    and getattr(inst, "engine", None) is not None
                ):
                    waits = list(si.on_wait)
                    cls = (
                        mybir.InstDrain
                        if isinstance(inst, mybir.InstDrain)
                        else mybir.InstNoOp
                    )
                    for i, w in enumerate(waits[:-1]):
                        extra = cls(
                            name=f"{inst.name}_w{i}",
                            engine=inst.engine,
                            sync_info=mybir.SyncInfo(on_wait=[w], on_update=[]),
                            bass_nofuse=True,
                        )
                        nc.register_instruction(extra)
                        out.append(extra)
                    si.on_wait = waits[-1:]
                out.append(inst)
            blk.instructions = out


# test/debug hook: kernel() appends the BassKernelResults of each phase here
LAST_RESULTS = []


def _build_phase1(ka):
    """SPMD program: priors for `ka` adapter slots per core.

    inputs : xw  [ka, 600, 856] f16  (cols 0:256 = x^T slice, 256:856 = W)
    output : pri  [ka, 2, 128, 600] f16  (priors [b, n*d], b in 2 chunks)
             vsum [2, 128, 600] f32      (sum of this core's adapters' priors)
    """
    nc = bass.Bass()
    xw = nc.declare_dram_parameter("xw", [ka, INCH, B + ND], F16, isOutput=False)
    pri = nc.declare_dram_parameter("pri", [ka, 2, 128, ND], F16, isOutput=True)
    vsum = nc.declare_dram_parameter("vsum", [2, 128, ND], F32, isOutput=True)

    with tile.TileContext(nc) as tc:
        with (
            tc.tile_pool(name="xt", bufs=1) as xt_pool,
            tc.tile_pool(name="ob", bufs=2 * ka) as ob_pool,
            tc.tile_pool(name="vs", bufs=2) as vs_pool,
            tc.tile_pool(name="ps", bufs=2, space="PSUM") as ps_pool,
        ):
            # ci-OUTER schedule: as soon as chunk ci's DMA lands, its four
            # (bc, gi) matmuls accumulate into the adapter's 4 live PSUM
            # tiles; junk matmuls absorb each chunk-DMA tick into PE's clock.
            ps_junk = ps_pool.tile([1, 1], F32, tag="pjunk", bufs=1)
            osbs = [[None] * 2 for _ in range(ka)]
            vsts = []
            for k in range(ka):
                pss = [
                    ps_pool.tile(
                        [128, ND // 2], F32, tag=f"ps{bc}_{gi}",
                        name=f"ps{k}_{bc}_{gi}", bufs=1,
                    )
                    for bc in range(2)
                    for gi in range(2)
                ]
                for ci, (c0, cs) in enumerate(_K_CHUNKS):
                    xw_t = xt_pool.tile(
                        [cs, B + ND], F16, tag=f"xw{k}_{ci}", name=f"xw{k}_{ci}"
                    )
                    nc.sync.dma_start(out=xw_t[:, :], in_=xw[k, c0 : c0 + cs, :])
                    nc.tensor.matmul(
                        ps_junk[:, :], xw_t[0:1, 0:1], xw_t[0:1, 0:1],
                        start=True, stop=True,
                    )
                    for bc in range(2):
                        for gi in range(2):
                            nc.tensor.matmul(
                                pss[2 * bc + gi][:, :],
                                xw_t[:, bc * 128 : (bc + 1) * 128],
                                xw_t[:, B + gi * 300 : B + (gi + 1) * 300],
                                start=(ci == 0),
                                stop=(ci == len(_K_CHUNKS) - 1),
                            )
                # batched per-adapter osb [p, (bc, nd)]: ONE SWDGE store per
                # adapter (a HWDGE store cannot work: every HWDGE DMA carries
                # a queue-ring wait, leaving no slot for the data wait)
                osb = ob_pool.tile([128, 2 * ND], F16, tag="osb")
                for bc in range(2):
                    nc.vector.tensor_copy(
                        osb[:, bc * ND : bc * ND + 300], pss[2 * bc][:, :]
                    )
                    nc.scalar.copy(
                        osb[:, bc * ND + 300 : (bc + 1) * ND], pss[2 * bc + 1][:, :]
                    )
                oab = ob_pool.tile([1, 4], F16, tag=f"oab{k}", bufs=1)
                nc.gpsimd.tensor_copy(oab[:, :], osb[0:1, 299 : 2 * ND : 300])
                nc.gpsimd.dma_start(
                    out=pri[k, :, :, :].rearrange("b p d -> p b d"),
                    in_=osb[:, :].rearrange("p (b d) -> p b d", b=2),
                )
                for bc in range(2):
                    osbs[k][bc] = osb[:, bc * ND : (bc + 1) * ND]
                    # iteration-1 vote partial accumulates as adapters finish
                    if ka > 1 and k == 1:
                        vst = vs_pool.tile([128, ND], F32, tag=f"vst{bc}", bufs=1)
                        nc.vector.scalar_tensor_tensor(
                            out=vst[:, :], in0=osbs[0][bc][:, :], scalar=1.0,
                            in1=osbs[1][bc][:, :], op0=ALU.mult, op1=ALU.add,
                        )
                        vsts.append(vst)
                    elif ka > 2 and k >= 2:
                        vst = vsts[bc]
                        nc.vector.scalar_tensor_tensor(
                            out=vst[:, :], in0=osbs[k][bc][:, :], scalar=1.0,
                            in1=vst[:, :], op0=ALU.mult, op1=ALU.add,
                        )
            # single SWDGE store for both vsum halves (one descriptor-gen)
            if ka == 1:
                vsts = []
                for bc in range(2):
                    vst = vs_pool.tile([128, ND], F32, tag=f"vst{bc}", bufs=1)
                    nc.vector.tensor_copy(vst[:, :], osbs[0][bc][:, :])
                    vsts.append(vst)
            vab = vs_pool.tile([1, 2], F32, tag="vab", bufs=1)
            nc.gpsimd.tensor_copy(vab[:, 0:1], vsts[0][0:1, ND - 1 :])
            nc.gpsimd.tensor_copy(vab[:, 1:2], vsts[1][0:1, ND - 1 :])
            nc.gpsimd.dma_start(out=vsum[0, :, :], in_=vsts[0][:, :])
            nc.gpsimd.dma_start(out=vsum[1, :, :], in_=vsts[1][:, :])
            _ = vab
    return nc


def _build_phase2(A):
    """SPMD program: routing for 96 (n,b) pairs + output projection per core.

    inputs : pri2 [96, A*200] f16  (priors for this core's pairs)
             lgi  [96, A] f32      (iteration-1 logits, host-computed)
             e2i  [96, A] f32      (softmax numerator of lgi, host-computed)
             di2i [96, 1] f32      (1/sum(e2i), host-computed)
             lwg  [3, 768] f16     (larger_w * gate, transposed)
    output : outc [6400, 768] f16

    Iteration 1 (agreement of the host-reduced vote sum vs1 with every
    prior + its softmax) runs on the host between the phases — it needs
    only phase-1 outputs, is 0.05% of the FLOPs, and removes ~12us from
    the device critical path.
    """
    nc = bass.Bass()
    pri2 = nc.declare_dram_parameter("pri2", [PP, A * D], F16, isOutput=False)
    lgi = nc.declare_dram_parameter("lgi", [PP, A], F32, isOutput=False)
    e2i = nc.declare_dram_parameter("e2i", [PP, A], F32, isOutput=False)
    di2i = nc.declare_dram_parameter("di2i", [PP, 1], F32, isOutput=False)
    lwg = nc.declare_dram_parameter("lwg", [CAPS, M], F16, isOutput=False)
    outc = nc.declare_dram_parameter("outc", [ROWS_PER_CORE, M], F16, isOutput=True)
    vd = nc.dram_tensor("vd", [PP, D], F16)  # flat vote staging

    uid = [0]

    with tile.TileContext(nc) as tc:
        with (
            tc.tile_pool(name="ps", bufs=2, space="PSUM") as ps_pool,
            tc.tile_pool(name="ob", bufs=2) as ob_pool,
            tc.tile_pool(name="sb", bufs=1) as sb,
        ):
            def fresh(shape, dtype=F32, pfx="t"):
                uid[0] += 1
                return sb.tile(shape, dtype, tag=f"{pfx}{uid[0]}", name=f"{pfx}{uid[0]}")

            # ---- input DMAs: first P slice + softmax pieces lead their
            # queues so vote2's chain starts as early as possible ----
            P = sb.tile([PP, A * D], F16, tag="P")
            Pv = P[:, :].rearrange("p (k d) -> p k d", k=A)
            KSL = (A + 2) // 3  # 7 slices of <=3 adapters
            e2_t = sb.tile([PP, A], F32, tag="e2")
            nc.scalar.dma_start(out=e2_t[:, :], in_=e2i[:, :])
            for si in range(KSL):
                k0 = si * 3
                k1 = min(k0 + 3, A)
                eng = nc.sync if si % 2 == 0 else nc.scalar
                eng.dma_start(
                    out=P[:, k0 * D : k1 * D], in_=pri2[:, k0 * D : k1 * D]
                )
            di2_t = sb.tile([PP, 1], F32, tag="di2")
            nc.scalar.dma_start(out=di2_t[:, :], in_=di2i[:, :])
            lgi_t = sb.tile([PP, A], F32, tag="lgi")
            nc.scalar.dma_start(out=lgi_t[:, :], in_=lgi[:, :])
            lwg_t = sb.tile([CAPS, M], F16, tag="lwg")
            nc.scalar.dma_start(out=lwg_t[:, :], in_=lwg[:, :])

            # ---- per-iteration helpers (all big element-wise work on DVE;
            # walrus rejects TensorScalarPtr on Pool) ----
            junk_dve = [fresh([PP, D], F32, "jd") for _ in range(2)]

            def agreement(v_t, tag):
                """aT[:, k] = sum_d P[:, k, :] * v_t (fused mult+reduce)."""
                aT = fresh([PP, A], F32, f"aT{tag}")
                for k in range(A):
                    nc.vector.scalar_tensor_tensor(
                        out=junk_dve[k % 2][:, :], in0=Pv[:, k, :],
                        scalar=1.0, in1=v_t[:, :],
                        op0=ALU.mult, op1=ALU.mult,
                        accum_out=aT[:, k : k + 1],
                    )
                return aT

            def vote(w_t, tag, out_dtype=F32):
                """vs = sum_k w_t[:, k] * P[:, k, :], two interleaved chains."""
                vs = fresh([PP, D], out_dtype, f"vs{tag}")
                accs = []
                for ci in range(2):
                    acc_c = fresh([PP, D], F32, f"va{tag}{ci}")
                    nc.vector.tensor_scalar(
                        out=acc_c[:, :], in0=Pv[:, ci, :],
                        scalar1=w_t[:, ci : ci + 1], scalar2=None,
                        op0=ALU.mult,
                    )
                    accs.append(acc_c)
                for k in range(2, A):
                    c = k % 2
                    nc.vector.scalar_tensor_tensor(
                        out=accs[c][:, :], in0=Pv[:, k, :],
                        scalar=w_t[:, k : k + 1], in1=accs[c][:, :],
                        op0=ALU.mult, op1=ALU.add,
                    )
                nc.vector.tensor_tensor(
                    out=vs[:, :], in0=accs[0][:, :], in1=accs[1][:, :],
                    op=ALU.add,
                )
                return vs

            def softmax(logit, tag):
                """returns (e, dinv): e = exp(logit - max), dinv = 1/sum(e)."""
                rmax = fresh([PP, 1], F32, f"rmx{tag}")
                nmax = fresh([PP, 1], F32, f"nmx{tag}")
                e = fresh([PP, A], F32, f"e{tag}")
                dsum = fresh([PP, 1], F32, f"dsm{tag}")
                dinv = fresh([PP, 1], F32, f"dnv{tag}")
                nc.vector.tensor_reduce(rmax[:, :], logit[:, :], AX.X, ALU.max)
                nc.vector.tensor_scalar(
                    out=nmax[:, :], in0=rmax[:, :], scalar1=-1.0, scalar2=None,
                    op0=ALU.mult,
                )
                nc.scalar.activation(
                    e[:, :], logit[:, :], ACTF.Exp, bias=nmax[:, 0:1],
                    accum_out=dsum[:, 0:1],
                )
                nc.vector.reciprocal(dinv[:, :], dsum[:, :])
                return e, dinv

            def g_chain(v_t, dinv, sq_scale, g_extra, tag):
                """g = g_extra * sqrt(sq)/(1+sq), sq = sum((v_t*sq_scale)^2)
                or sum(v_t^2)*dinv^2. ACT square/ln/exp + DVE recip; the
                squash factor applied to agreements instead of the vote.
                g_extra is a float or a [PP,1] AP (the dinv)."""
                jnk = fresh([PP, D], F32, f"gj{tag}")
                sq = fresh([PP, 1], F32, f"sq{tag}")
                if dinv is None:
                    nc.scalar.activation(
                        jnk[:, :], v_t[:, :], ACTF.Square, scale=sq_scale,
                        accum_out=sq[:, 0:1],
                    )
                else:
                    ssq = fresh([PP, 1], F32, f"ssq{tag}")
                    nc.scalar.activation(
                        jnk[:, :], v_t[:, :], ACTF.Square, accum_out=ssq[:, 0:1]
                    )
                    nc.vector.scalar_tensor_tensor(
                        out=sq[:, :], in0=ssq[:, :], scalar=dinv[:, 0:1],
                        in1=dinv[:, :], op0=ALU.mult, op1=ALU.mult,
                    )
                lnv = fresh([PP, 1], F32, f"ln{tag}")
                nc.scalar.activation(lnv[:, :], sq[:, :], ACTF.Ln)
                rt = fresh([PP, 1], F32, f"rt{tag}")
                nc.scalar.activation(rt[:, :], lnv[:, :], ACTF.Exp, scale=0.5)
                sp = fresh([PP, 1], F32, f"sp{tag}")
                nc.vector.tensor_scalar(
                    out=sp[:, :], in0=sq[:, :], scalar1=1.0, scalar2=None,
                    op0=ALU.add,
                )
                rc = fresh([PP, 1], F32, f"rc{tag}")
                nc.vector.reciprocal(rc[:, :], sp[:, :])
                g = fresh([PP, 1], F32, f"g{tag}")
                if isinstance(g_extra, float):
                    nc.vector.scalar_tensor_tensor(
                        out=g[:, :], in0=rt[:, :], scalar=g_extra, in1=rc[:, :],
                        op0=ALU.mult, op1=ALU.mult,
                    )
                else:
                    nc.vector.scalar_tensor_tensor(
                        out=g[:, :], in0=rt[:, :], scalar=g_extra[:, 0:1],
                        in1=rc[:, :], op0=ALU.mult, op1=ALU.mult,
                    )
                return g

            # ---- iteration 2 (iteration 1 + softmax arrive from host) ----
            vs2 = vote(e2_t, "2")
            g2 = g_chain(vs2, di2_t, None, di2_t, "2")
            aT2 = agreement(vs2, "2")
            logit2 = fresh([PP, A], F32, "lg2")
            nc.vector.scalar_tensor_tensor(
                out=logit2[:, :], in0=aT2[:, :], scalar=g2[:, 0:1],
                in1=lgi_t[:, :], op0=ALU.mult, op1=ALU.add,
            )

            # ---- iteration 3: final vote. dinv3 folds into e3 (one tiny
            # [96,A] op) so the vote chain emits the normalized vote and the
            # combine writes f16 directly — no [96,200] ACT rescale on the
            # critical transition path ----
            e3, dinv3 = softmax(logit2, "3")
            p3 = fresh([PP, A], F32, "p3")
            nc.vector.tensor_scalar(
                out=p3[:, :], in0=e3[:, :], scalar1=dinv3[:, 0:1],
                scalar2=None, op0=ALU.mult,
            )
            vs3 = vote(p3, "3", out_dtype=F16)

            # ---- deinterleave the flat vote stream into u^T rows, in TWO
            # independent pair-halves so the first projection batches start
            # while the second half is still in flight: [48,200] -> [16,600]
            # partition regroup -> strided in-partition deinterleave ->
            # DRAM bounce -> uT row-halves. (SBUF->SBUF DMA cannot advance
            # src/dst partition indices independently, hence the bounce;
            # SWDGE + absorbers because HWDGE queues can't carry a data
            # wait on top of their ring wait.)
            uT = sb.tile([CAPS, ROWS_PER_CORE], F16, tag="uT")
            HP = PP // 2  # 48 pairs per half
            HQ = HP // CAPS  # 16 groups per half
            HR = ROWS_PER_CORE // 2  # 3200 rows per half
            # single chain (a direct stride-3 DRAM gather would need 19200
            # per-element descriptors — over the 16384 limit and ~6.5us of
            # SWDGE gen — so the vstack realignment stays)
            v3h = vs3  # already normalized f16 (dinv3 folded into e3)
            vab = fresh([1, 1], F16, "vab")
            nc.gpsimd.tensor_copy(vab[:, :], v3h[0:1, D - 1 : D])
            vstack = fresh([PP // CAPS, CAPS * D], F16, "vstk")
            nc.gpsimd.dma_start(
                out=vstack[:, :].rearrange("q (m d) -> q m d", m=CAPS),
                in_=v3h[:, :],
            )
            uT2 = fresh([PP // CAPS, CAPS * D], F16, "uT2")
            nc.vector.tensor_copy(
                uT2[:, :].rearrange("q (k jl) -> q k jl", k=CAPS),
                vstack[:, :].rearrange("q (jl k) -> q k jl", k=CAPS),
            )
            uab = fresh([1, 1], F16, "uab")
            nc.gpsimd.tensor_copy(uab[:, :], uT2[0:1, CAPS * D - 1 :])
            nc.gpsimd.dma_start(
                out=vd[:, :].rearrange("p d -> (p d)")
                .rearrange("(k x) -> k x", k=CAPS)
                .rearrange("k (q jl) -> q k jl", q=PP // CAPS),
                in_=uT2[:, :].rearrange("q (k jl) -> q k jl", k=CAPS),
            )
            nc.gpsimd.dma_start(
                out=uT[:, :],
                in_=vd[:, :].rearrange("p d -> (p d)")
                .rearrange("(k x) -> k x", k=CAPS),
            )

            # PE absorbers: junk matmuls ladder the uT-writer + lwg ticks
            # into PE's clock (dep tracking is byte-range based)
            ps_junk = ps_pool.tile([1, 1], F32, tag="pjunk", bufs=1)
            for labs in (lwg_t[0:1, 0:1], uT[0:1, 0:1], uT[0:3, 0:1]):
                nc.tensor.matmul(ps_junk[:, :], labs, labs, start=True, stop=True)

            # ---- projection: out[j, :] = uT[:, j].T @ lwg, fp16 store.
            # PSUM bufs=3 per half keep the PE running ahead of evacuation;
            # evacuation copies split DVE 4 / ACT 4 / Pool 2 per batch ----
            HM = M // 2
            BCH = 5
            # GPSIMD cannot read PSUM — evacuation alternates DVE/ACT only
            evacA = [nc.vector, nc.scalar, nc.vector, nc.scalar, nc.vector]
            evacB = [nc.scalar, nc.vector, nc.scalar, nc.vector, nc.scalar]
            last_ab = None
            for bt in range(JCH // BCH):
                if last_ab is not None:
                    # pull the previous Pool-absorber tick into DVE so a
                    # recycled slot's first copy carries only the store wait
                    s = fresh([1, 1], F16, "slv")
                    nc.vector.tensor_copy(s[:, :], last_ab[0:1, 0:1])
                osb = ob_pool.tile([128, BCH * M], F16, tag="osb", name="osb", bufs=3)
                for ji in range(BCH):
                    jc = bt * BCH + ji
                    js = jc * 128
                    co = ji * M
                    # single rotating psum tag, depth 7 (+1 junk bank = 8):
                    # lets the PE run ~3.5 chunks ahead of evacuation
                    psA = ps_pool.tile([128, HM], F32, tag="psAB", name="psA", bufs=7)
                    psB = ps_pool.tile([128, HM], F32, tag="psAB", name="psB", bufs=7)
                    nc.tensor.matmul(
                        psA[:, :], uT[:, js : js + 128], lwg_t[:, :HM],
                        start=True, stop=True,
                    )
                    nc.tensor.matmul(
                        psB[:, :], uT[:, js : js + 128], lwg_t[:, HM:],
                        start=True, stop=True,
                    )
                    if evacA[ji] is nc.scalar:
                        nc.scalar.copy(osb[:, co : co + HM], psA[:, :])
                    else:
                        evacA[ji].tensor_copy(osb[:, co : co + HM], psA[:, :])
                    if evacB[ji] is nc.scalar:
                        nc.scalar.copy(osb[:, co + HM : co + M], psB[:, :])
                    else:
                        evacB[ji].tensor_copy(osb[:, co + HM : co + M], psB[:, :])
                r0 = bt * BCH * 128
                ab = fresh([1, 2 * BCH], F16, "pba")
                nc.gpsimd.tensor_copy(
                    ab[:, :], osb[0:1, HM - 1 : BCH * M : HM]
                )
                nc.gpsimd.dma_start(
                    out=outc[r0 : r0 + BCH * 128, :].rearrange(
                        "(j p) m -> p j m", p=128
                    ),
                    in_=osb[:, :].rearrange("p (j m) -> p j m", j=BCH),
                )
                last_ab = ab
    return nc


def _get_programs(A, ka):
    key = (A, ka)
    if key not in _BUILD_CACHE:
        nc1, nc2 = _build_phase1(ka), _build_phase2(A)
        _split_multiwait_waits(nc1)
        _split_multiwait_waits(nc2)
        _BUILD_CACHE[key] = (nc1, nc2)
    return _BUILD_CACHE[key]


def kernel(t, x, s, route_weights, larger_w, larger_b, elarger, tsv):
    t = int(t)
    x = np.ascontiguousarray(np.asarray(x, np.float32))
    tsv_t = np.asarray(tsv, np.float32)[t]
    allowed = np.nonzero(tsv_t != 0)[0]
    assert np.all(tsv_t[allowed] == 1.0), "tsv gate values must be 1"
    A = len(allowed)
    ka = (A + NC - 1) // NC

    nc1, nc2 = _get_programs(A, ka)

    # ---------- phase 1: priors, expert-parallel ----------
    rw = np.asarray(route_weights, np.float32)
    in1 = []
    for c in range(NC):
        xw_c = np.zeros((ka, INCH, B + ND), np.float16)
        for j in range(ka):
            g = c * ka + j
            if g < A:
                k = allowed[g]
                xw_c[j, :, :B] = x[:, k, :].T
                xw_c[j, :, B:] = rw[k].transpose(1, 0, 2).reshape(INCH, ND)
        in1.append({"xw": xw_c})
    res1 = run_bass_kernel_spmd(nc1, in1, list(range(NC)))
    LAST_RESULTS.append(res1)

    # priors_full[k, b, n, d] — stays f16 (phase-2 reads it as f16)
    priors_full = np.zeros((A, B, CAPS, D), np.float16)
    vs_full = np.zeros((B, ND), np.float32)
    for c in range(NC):
        pri = res1.results[c]["pri"]  # [ka, 2, 128, 600] f16
        vs_full += res1.results[c]["vsum"].reshape(B, ND)
        for j in range(ka):
            g = c * ka + j
            if g < A:
                priors_full[g] = pri[j].reshape(B, CAPS, D)

    # ---------- phase 2: routing + projection, pair-parallel ----------
    g_gate = 1.0 / (
        1.0 + np.exp(-(np.float32(s[0]) * np.asarray(elarger, np.float32)[t]))
    )
    lwg_f = np.asarray(larger_w, np.float32) * g_gate[:, None]  # [768, 3]
    bg = np.asarray(larger_b, np.float32) * g_gate  # [768]
    assert not np.any(bg), "nonzero larger_b not supported by this build"
    lwg_16 = np.ascontiguousarray(lwg_f.T.astype(np.float16))  # [3, 768]

    # iteration 1 on host: logit1 = g1 * <P_k, vs1>, plus its softmax pieces
    vs_v = vs_full.reshape(B, CAPS, D)
    inv_a = np.float32(1.0 / A)
    in2 = []
    for c in range(NC):
        sidx = np.arange(c * PP, (c + 1) * PP)
        nv, bv = sidx // B, sidx % B
        P2 = priors_full[:, bv, nv, :].transpose(1, 0, 2)  # [96, A, 200] f16
        vsp = vs_v[bv, nv, :]  # [96, 200] f32
        sq1 = (vsp * vsp).sum(-1) * inv_a * inv_a
        g1 = inv_a * np.sqrt(sq1) / (1.0 + sq1)
        aT1 = np.einsum("skd,sd->sk", P2.astype(np.float32), vsp)
        logit1 = (g1[:, None] * aT1).astype(np.float32)
        e2 = np.exp(logit1 - logit1.max(-1, keepdims=True))
        di2 = (1.0 / e2.sum(-1, keepdims=True)).astype(np.float32)
        in2.append(
            {
                "pri2": np.ascontiguousarray(P2.reshape(PP, A * D)),
                "lgi": logit1,
                "e2i": e2.astype(np.float32),
                "di2i": di2,
                "lwg": lwg_16,
            }
        )
    res2 = run_bass_kernel_spmd(nc2, in2, list(range(NC)))
    LAST_RESULTS.append(res2)

    out = np.concatenate(
        [res2.results[c]["outc"].astype(np.float32) for c in range(NC)], axis=0
    )
    return out.reshape(B, D, M)


# revision 47
# speedup vs baseline: 1.0225x; 1.0225x over previous
"""Trainium2 Bass kernel for nn_CapsuleLayerTSV (capsule routing over 40 adapters).

Strategy (8 NeuronCores, two SPMD NEFFs, no collectives), all fp16 on the wire
(11-bit mantissa ~ f32r precision; routing softmax is too sensitive for bf16 —
measured 8e-2 rel err with bf16 priors vs 1e-3 with fp16):

  Phase 1 (expert-parallel): allowed adapters (tsv[t] != 0) sharded 3-per-core.
    Each core computes priors[b, n*d] = x[:, k, :] @ W[k] as fp16 matmuls
    (1 cyc/row) with f32 PSUM accumulate, chunk-outer so the PE chases the
    DMA stream. Outputs priors in fp16 (halves the store); a per-core f32
    adapter-sum partial (iteration-1 vote) rides on the DVE.
  Host: reassemble priors (fp16 -> f32), all-reduce the vote partials,
    re-shard by the output's flat row space (output row r uses flat vote
    elements 3r..3r+2; core c gets pairs s in [96c, 96c+96), s = n*256 + b).
  Phase 2 (pair-parallel): 3-iteration dynamic routing for 96 pairs per core,
    then u[6400,3] @ lwg[3,768] with the sigmoid gate folded in on host.
    - squash factored into a per-pair scalar: <P_k, squash(v)> =
      g * <P_k, v> with g = dinv*sqrt(sq)/(1+sq), so the squashed vote is
      never materialized and agreements run on the raw vote.
    - sqrt via exp(0.5*ln): keeps ACT on the single natural_log_exp table
      (square/ln/exp/copy) -- zero act-table reloads (4x 1.28us saved).
    - tsv values on allowed adapters are identically 1 (tril of ones), so
      all tsv multiplies drop out (asserted on host).
    - agreement + vote run on DVE (walrus rejects TensorScalarPtr on Pool,
      and Pool TT+reduce pairs are slower than DVE's fused op; ACT runs the
      square/ln/exp scalar chain concurrently).
    - fp16 output store (halves the 19.7MB/core store to 9.8MB ~ 28us at
      the ~355GB/s per-core roofline); host upcasts to f32.
    - vote deinterleave to u^T entirely in SBUF (no DRAM bounce).
"""

import sys

sys.path.insert(0, "/opt/trn_rl_repo")

import numpy as np

import concourse.bass as bass
import concourse.mybir as mybir
import concourse.tile as tile
from concourse.bass_utils import run_bass_kernel_spmd

F32 = mybir.dt.float32
F16 = mybir.dt.float16
AX = mybir.AxisListType
ALU = mybir.AluOpType
ACTF = mybir.ActivationFunctionType

NC = 8
B = 256
ADP = 40
CAPS = 3
INCH = 600
D = 200
M = 768
ND = CAPS * D  # 600
PP = CAPS * B // NC  # 96 (n,b) pairs per core in phase 2
ROWS_PER_CORE = PP * D // CAPS  # 6400 output rows per core
JCH = ROWS_PER_CORE // 128  # 50 j-chunks

_K_CHUNKS = [(0, 128), (128, 128), (256, 128), (384, 128), (512, 88)]

_BUILD_CACHE = {}


def _split_multiwait_waits(nc):
    """walrus caps sync-waits at ONE per instruction. For instructions executed
    by an in-order engine sequencer (everything except queue-executed DMAs),
    splitting the wait list across preceding 1-wait NoOps/Drains on the same
    engine is semantics-preserving."""
    for fn in nc.m.functions:
        for blk in fn.blocks:
            out = []
            for inst in blk.instructions:
                si = getattr(inst, "sync_info", None)
                if (
                    si is not None
                    and si.on_wait
                    and len(si.on_wait) > 1
                    and not isinstance(inst, mybir.InstDMACopy)
                    and getattr(inst, "engine", None) is not None
                ):
                    waits = list(si.on_wait)
                    cls = (
                        mybir.InstDrain
                        if isinstance(inst, mybir.InstDrain)
                        else mybir.InstNoOp
                    )
                    for i, w in enumerate(waits[:-1]):
                        extra = cls(
                            name=f"{inst.name}_w{i}",
                            engine=inst.engine,
                            sync_info=mybir.SyncInfo(on_wait=[w], on_update=[]),
                            bass_nofuse=True,
                        )
                        nc.register_instruction(extra)
                        out.append(extra)
                    si.on_wait = waits[-1:]
                out.append(inst)
            blk.instructions = out


# test/debug hook: kernel() appends the BassKernelResults of each phase here
LAST_RESULTS = []


def _build_phase1(ka):
    """SPMD program: priors for `ka` adapter slots per core.

    inputs : xw  [ka, 600, 856] f16  (cols 0:256 = x^T slice, 256:856 = W)
    output : pri  [ka, 2, 128, 600] f16  (priors [b, n*d], b in 2 chunks)
             vsum [2, 128, 600] f32      (sum of this core's adapters' priors)
    """
    nc = bass.Bass()
    xw = nc.declare_dram_parameter("xw", [ka, INCH, B + ND], F16, isOutput=False)
    pri = nc.declare_dram_parameter("pri", [ka, 2, 128, ND], F16, isOutput=True)
    vsum = nc.declare_dram_parameter("vsum", [2, 128, ND], F32, isOutput=True)

    with tile.TileContext(nc) as tc:
        with (
            tc.tile_pool(name="xt", bufs=1) as xt_pool,
            tc.tile_pool(name="ob", bufs=2 * ka) as ob_pool,
            tc.tile_pool(name="vs", bufs=2) as vs_pool,
            tc.tile_pool(name="ps", bufs=2, space="PSUM") as ps_pool,
        ):
            # ci-OUTER schedule: as soon as chunk ci's DMA lands, its four
            # (bc, gi) matmuls accumulate into the adapter's 4 live PSUM
            # tiles, chasing the DMA stream chunk-by-chunk.
            osbs = [[None] * 2 for _ in range(ka)]
            vsts = []
            for k in range(ka):
                pss = [
                    ps_pool.tile(
                        [128, ND // 2], F32, tag=f"ps{bc}_{gi}",
                        name=f"ps{k}_{bc}_{gi}", bufs=1,
                    )
                    for bc in range(2)
                    for gi in range(2)
                ]
                for ci, (c0, cs) in enumerate(_K_CHUNKS):
                    xw_t = xt_pool.tile(
                        [cs, B + ND], F16, tag=f"xw{k}_{ci}", name=f"xw{k}_{ci}"
                    )
                    nc.sync.dma_start(out=xw_t[:, :], in_=xw[k, c0 : c0 + cs, :])
                    # (no junk absorber matmul: _split_multiwait_waits lets the
                    # real matmuls carry the DMA tick + PSUM-release waits)
                    for bc in range(2):
                        for gi in range(2):
                            nc.tensor.matmul(
                                pss[2 * bc + gi][:, :],
                                xw_t[:, bc * 128 : (bc + 1) * 128],
                                xw_t[:, B + gi * 300 : B + (gi + 1) * 300],
                                start=(ci == 0),
                                stop=(ci == len(_K_CHUNKS) - 1),
                            )
                # batched per-adapter osb [p, (bc, nd)]: ONE SWDGE store per
                # adapter (a HWDGE store cannot work: every HWDGE DMA carries
                # a queue-ring wait, leaving no slot for the data wait)
                osb = ob_pool.tile([128, 2 * ND], F16, tag="osb")
                for bc in range(2):
                    nc.vector.tensor_copy(
                        osb[:, bc * ND : bc * ND + 300], pss[2 * bc][:, :]
                    )
                    nc.scalar.copy(
                        osb[:, bc * ND + 300 : (bc + 1) * ND], pss[2 * bc + 1][:, :]
                    )
                oab = ob_pool.tile([1, 4], F16, tag=f"oab{k}", bufs=1)
                nc.gpsimd.tensor_copy(oab[:, :], osb[0:1, 299 : 2 * ND : 300])
                nc.gpsimd.dma_start(
                    out=pri[k, :, :, :].rearrange("b p d -> p b d"),
                    in_=osb[:, :].rearrange("p (b d) -> p b d", b=2),
                )
                for bc in range(2):
                    osbs[k][bc] = osb[:, bc * ND : (bc + 1) * ND]
                    # iteration-1 vote partial accumulates as adapters finish
                    if ka > 1 and k == 1:
                        vst = vs_pool.tile([128, ND], F32, tag=f"vst{bc}", bufs=1)
                        nc.vector.scalar_tensor_tensor(
                            out=vst[:, :], in0=osbs[0][bc][:, :], scalar=1.0,
                            in1=osbs[1][bc][:, :], op0=ALU.mult, op1=ALU.add,
                        )
                        vsts.append(vst)
                    elif ka > 2 and k >= 2:
                        vst = vsts[bc]
                        nc.vector.scalar_tensor_tensor(
                            out=vst[:, :], in0=osbs[k][bc][:, :], scalar=1.0,
                            in1=vst[:, :], op0=ALU.mult, op1=ALU.add,
                        )
            # single SWDGE store for both vsum halves (one descriptor-gen)
            if ka == 1:
                vsts = []
                for bc in range(2):
                    vst = vs_pool.tile([128, ND], F32, tag=f"vst{bc}", bufs=1)
                    nc.vector.tensor_copy(vst[:, :], osbs[0][bc][:, :])
                    vsts.append(vst)
            vab = vs_pool.tile([1, 2], F32, tag="vab", bufs=1)
            nc.gpsimd.tensor_copy(vab[:, 0:1], vsts[0][0:1, ND - 1 :])
            nc.gpsimd.tensor_copy(vab[:, 1:2], vsts[1][0:1, ND - 1 :])
            nc.gpsimd.dma_start(out=vsum[0, :, :], in_=vsts[0][:, :])
            nc.gpsimd.dma_start(out=vsum[1, :, :], in_=vsts[1][:, :])
            _ = vab
    return nc


def _build_phase2(A):
    """SPMD program: routing for 96 (n,b) pairs + output projection per core.

    inputs : pri2 [96, A*200] f16  (priors for this core's pairs)
             lgi  [96, A] f32      (iteration-1 logits, host-computed)
             e2i  [96, A] f32      (softmax numerator of lgi, host-computed)
             di2i [96, 1] f32      (1/sum(e2i), host-computed)
             lwg  [3, 768] f16     (larger_w * gate, transposed)
    output : outc [6400, 768] f16

    Iteration 1 (agreement of the host-reduced vote sum vs1 with every
    prior + its softmax) runs on the host between the phases — it needs
    only phase-1 outputs, is 0.05% of the FLOPs, and removes ~12us from
    the device critical path.
    """
    nc = bass.Bass()
    pri2 = nc.declare_dram_parameter("pri2", [PP, A * D], F16, isOutput=False)
    lgi = nc.declare_dram_parameter("lgi", [PP, A], F32, isOutput=False)
    e2i = nc.declare_dram_parameter("e2i", [PP, A], F32, isOutput=False)
    di2i = nc.declare_dram_parameter("di2i", [PP, 1], F32, isOutput=False)
    lwg = nc.declare_dram_parameter("lwg", [CAPS, M], F16, isOutput=False)
    outc = nc.declare_dram_parameter("outc", [ROWS_PER_CORE, M], F16, isOutput=True)
    vd = nc.dram_tensor("vd", [PP, D], F16)  # flat vote staging

    uid = [0]

    with tile.TileContext(nc) as tc:
        with (
            tc.tile_pool(name="ps", bufs=2, space="PSUM") as ps_pool,
            tc.tile_pool(name="ob", bufs=2) as ob_pool,
            tc.tile_pool(name="sb", bufs=1) as sb,
        ):
            def fresh(shape, dtype=F32, pfx="t"):
                uid[0] += 1
                return sb.tile(shape, dtype, tag=f"{pfx}{uid[0]}", name=f"{pfx}{uid[0]}")

            # ---- input DMAs: first P slice + softmax pieces lead their
            # queues so vote2's chain starts as early as possible ----
            P = sb.tile([PP, A * D], F16, tag="P")
            Pv = P[:, :].rearrange("p (k d) -> p k d", k=A)
            KSL = (A + 2) // 3  # 7 slices of <=3 adapters
            e2_t = sb.tile([PP, A], F32, tag="e2")
            nc.scalar.dma_start(out=e2_t[:, :], in_=e2i[:, :])
            for si in range(KSL):
                k0 = si * 3
                k1 = min(k0 + 3, A)
                eng = nc.sync if si % 2 == 0 else nc.scalar
                eng.dma_start(
                    out=P[:, k0 * D : k1 * D], in_=pri2[:, k0 * D : k1 * D]
                )
            di2_t = sb.tile([PP, 1], F32, tag="di2")
            nc.scalar.dma_start(out=di2_t[:, :], in_=di2i[:, :])
            lgi_t = sb.tile([PP, A], F32, tag="lgi")
            nc.scalar.dma_start(out=lgi_t[:, :], in_=lgi[:, :])
            lwg_t = sb.tile([CAPS, M], F16, tag="lwg")
            nc.scalar.dma_start(out=lwg_t[:, :], in_=lwg[:, :])

            # ---- per-iteration helpers (all big element-wise work on DVE;
            # walrus rejects TensorScalarPtr on Pool) ----
            junk_dve = [fresh([PP, D], F32, "jd") for _ in range(2)]

            def agreement(v_t, tag):
                """aT[:, k] = sum_d P[:, k, :] * v_t (fused mult+reduce)."""
                aT = fresh([PP, A], F32, f"aT{tag}")
                for k in range(A):
                    nc.vector.scalar_tensor_tensor(
                        out=junk_dve[k % 2][:, :], in0=Pv[:, k, :],
                        scalar=1.0, in1=v_t[:, :],
                        op0=ALU.mult, op1=ALU.mult,
                        accum_out=aT[:, k : k + 1],
                    )
                return aT

            def vote(w_t, tag, out_dtype=F32):
                """vs = sum_k w_t[:, k] * P[:, k, :], two interleaved chains."""
                vs = fresh([PP, D], out_dtype, f"vs{tag}")
                accs = []
                for ci in range(2):
                    acc_c = fresh([PP, D], F32, f"va{tag}{ci}")
                    nc.vector.tensor_scalar(
                        out=acc_c[:, :], in0=Pv[:, ci, :],
                        scalar1=w_t[:, ci : ci + 1], scalar2=None,
                        op0=ALU.mult,
                    )
                    accs.append(acc_c)
                for k in range(2, A):
                    c = k % 2
                    nc.vector.scalar_tensor_tensor(
                        out=accs[c][:, :], in0=Pv[:, k, :],
                        scalar=w_t[:, k : k + 1], in1=accs[c][:, :],
                        op0=ALU.mult, op1=ALU.add,
                    )
                nc.vector.tensor_tensor(
                    out=vs[:, :], in0=accs[0][:, :], in1=accs[1][:, :],
                    op=ALU.add,
                )
                return vs

            def softmax(logit, tag):
                """returns (e, dinv): e = exp(logit - max), dinv = 1/sum(e)."""
                rmax = fresh([PP, 1], F32, f"rmx{tag}")
                nmax = fresh([PP, 1], F32, f"nmx{tag}")
                e = fresh([PP, A], F32, f"e{tag}")
                dsum = fresh([PP, 1], F32, f"dsm{tag}")
                dinv = fresh([PP, 1], F32, f"dnv{tag}")
                nc.vector.tensor_reduce(rmax[:, :], logit[:, :], AX.X, ALU.max)
                nc.vector.tensor_scalar(
                    out=nmax[:, :], in0=rmax[:, :], scalar1=-1.0, scalar2=None,
                    op0=ALU.mult,
                )
                nc.scalar.activation(
                    e[:, :], logit[:, :], ACTF.Exp, bias=nmax[:, 0:1],
                    accum_out=dsum[:, 0:1],
                )
                nc.vector.reciprocal(dinv[:, :], dsum[:, :])
                return e, dinv

            def g_chain(v_t, dinv, sq_scale, g_extra, tag):
                """g = g_extra * sqrt(sq)/(1+sq), sq = sum((v_t*sq_scale)^2)
                or sum(v_t^2)*dinv^2. ACT square/ln/exp + DVE recip; the
                squash factor applied to agreements instead of the vote.
                g_extra is a float or a [PP,1] AP (the dinv)."""
                jnk = fresh([PP, D], F32, f"gj{tag}")
                sq = fresh([PP, 1], F32, f"sq{tag}")
                if dinv is None:
                    nc.scalar.activation(
                        jnk[:, :], v_t[:, :], ACTF.Square, scale=sq_scale,
                        accum_out=sq[:, 0:1],
                    )
                else:
                    ssq = fresh([PP, 1], F32, f"ssq{tag}")
                    nc.scalar.activation(
                        jnk[:, :], v_t[:, :], ACTF.Square, accum_out=ssq[:, 0:1]
                    )
                    nc.vector.scalar_tensor_tensor(
                        out=sq[:, :], in0=ssq[:, :], scalar=dinv[:, 0:1],
                        in1=dinv[:, :], op0=ALU.mult, op1=ALU.mult,
                    )
                lnv = fresh([PP, 1], F32, f"ln{tag}")
                nc.scalar.activation(lnv[:, :], sq[:, :], ACTF.Ln)
                rt = fresh([PP, 1], F32, f"rt{tag}")
                nc.scalar.activation(rt[:, :], lnv[:, :], ACTF.Exp, scale=0.5)
                sp = fresh([PP, 1], F32, f"sp{tag}")
                nc.vector.tensor_scalar(
                    out=sp[:, :], in0=sq[:, :], scalar1=1.0, scalar2=None,
                    op0=ALU.add,
                )
                rc = fresh([PP, 1], F32, f"rc{tag}")
                nc.vector.reciprocal(rc[:, :], sp[:, :])
                g = fresh([PP, 1], F32, f"g{tag}")
                if isinstance(g_extra, float):
                    nc.vector.scalar_tensor_tensor(
                        out=g[:, :], in0=rt[:, :], scalar=g_extra, in1=rc[:, :],
                        op0=ALU.mult, op1=ALU.mult,
                    )
                else:
                    nc.vector.scalar_tensor_tensor(
                        out=g[:, :], in0=rt[:, :], scalar=g_extra[:, 0:1],
                        in1=rc[:, :], op0=ALU.mult, op1=ALU.mult,
                    )
                return g

            # ---- iteration 2 (iteration 1 + softmax arrive from host) ----
            vs2 = vote(e2_t, "2")
            g2 = g_chain(vs2, di2_t, None, di2_t, "2")
            aT2 = agreement(vs2, "2")
            logit2 = fresh([PP, A], F32, "lg2")
            nc.vector.scalar_tensor_tensor(
                out=logit2[:, :], in0=aT2[:, :], scalar=g2[:, 0:1],
                in1=lgi_t[:, :], op0=ALU.mult, op1=ALU.add,
            )

            # ---- iteration 3: final vote. dinv3 folds into e3 (one tiny
            # [96,A] op) so the vote chain emits the normalized vote and the
            # combine writes f16 directly — no [96,200] ACT rescale on the
            # critical transition path ----
            e3, dinv3 = softmax(logit2, "3")
            p3 = fresh([PP, A], F32, "p3")
            nc.vector.tensor_scalar(
                out=p3[:, :], in0=e3[:, :], scalar1=dinv3[:, 0:1],
                scalar2=None, op0=ALU.mult,
            )
            vs3 = vote(p3, "3", out_dtype=F16)

            # ---- deinterleave the flat vote stream into u^T rows, in TWO
            # independent pair-halves so the first projection batches start
            # while the second half is still in flight: [48,200] -> [16,600]
            # partition regroup -> strided in-partition deinterleave ->
            # DRAM bounce -> uT row-halves. (SBUF->SBUF DMA cannot advance
            # src/dst partition indices independently, hence the bounce;
            # SWDGE + absorbers because HWDGE queues can't carry a data
            # wait on top of their ring wait.)
            uT = sb.tile([CAPS, ROWS_PER_CORE], F16, tag="uT")
            HP = PP // 2  # 48 pairs per half
            HQ = HP // CAPS  # 16 groups per half
            HR = ROWS_PER_CORE // 2  # 3200 rows per half
            # single chain (a direct stride-3 DRAM gather would need 19200
            # per-element descriptors — over the 16384 limit and ~6.5us of
            # SWDGE gen — so the vstack realignment stays)
            v3h = vs3  # already normalized f16 (dinv3 folded into e3)
            vab = fresh([1, 1], F16, "vab")
            nc.gpsimd.tensor_copy(vab[:, :], v3h[0:1, D - 1 : D])
            vstack = fresh([PP // CAPS, CAPS * D], F16, "vstk")
            nc.gpsimd.dma_start(
                out=vstack[:, :].rearrange("q (m d) -> q m d", m=CAPS),
                in_=v3h[:, :],
            )
            uT2 = fresh([PP // CAPS, CAPS * D], F16, "uT2")
            nc.vector.tensor_copy(
                uT2[:, :].rearrange("q (k jl) -> q k jl", k=CAPS),
                vstack[:, :].rearrange("q (jl k) -> q k jl", k=CAPS),
            )
            uab = fresh([1, 1], F16, "uab")
            nc.gpsimd.tensor_copy(uab[:, :], uT2[0:1, CAPS * D - 1 :])
            nc.gpsimd.dma_start(
                out=vd[:, :].rearrange("p d -> (p d)")
                .rearrange("(k x) -> k x", k=CAPS)
                .rearrange("k (q jl) -> q k jl", q=PP // CAPS),
                in_=uT2[:, :].rearrange("q (k jl) -> q k jl", k=CAPS),
            )
            nc.gpsimd.dma_start(
                out=uT[:, :],
                in_=vd[:, :].rearrange("p d -> (p d)")
                .rearrange("(k x) -> k x", k=CAPS),
            )

            # PE absorbers: junk matmuls ladder the uT-writer + lwg ticks
            # into PE's clock (dep tracking is byte-range based)
            ps_junk = ps_pool.tile([1, 1], F32, tag="pjunk", bufs=1)
            for labs in (lwg_t[0:1, 0:1], uT[0:1, 0:1], uT[0:3, 0:1]):
                nc.tensor.matmul(ps_junk[:, :], labs, labs, start=True, stop=True)

            # ---- projection: out[j, :] = uT[:, j].T @ lwg, fp16 store.
            # PSUM bufs=3 per half keep the PE running ahead of evacuation;
            # evacuation copies split DVE 4 / ACT 4 / Pool 2 per batch ----
            HM = M // 2
            # GPSIMD cannot read PSUM — evacuation alternates DVE/ACT only
            evacA = [nc.vector, nc.scalar, nc.vector, nc.scalar, nc.vector]
            evacB = [nc.scalar, nc.vector, nc.scalar, nc.vector, nc.scalar]
            # last batch split 3+2 so the final store (and its end-of-kernel
            # drain) is shorter
            batches = [5] * (JCH // 5 - 1) + [3, 2]
            last_ab = None
            jc0 = 0
            for bt, BCH in enumerate(batches):
                if last_ab is not None:
                    # pull the previous Pool-absorber tick into DVE so a
                    # recycled slot's first copy carries only the store wait
                    s = fresh([1, 1], F16, "slv")
                    nc.vector.tensor_copy(s[:, :], last_ab[0:1, 0:1])
                osb = ob_pool.tile(
                    [128, BCH * M], F16, tag=f"osb{BCH}", name="osb", bufs=3
                )
                for ji in range(BCH):
                    jc = jc0 + ji
                    js = jc * 128
                    co = ji * M
                    # single rotating psum tag, depth 7 (+1 junk bank = 8):
                    # lets the PE run ~3.5 chunks ahead of evacuation
                    psA = ps_pool.tile([128, HM], F32, tag="psAB", name="psA", bufs=7)
                    psB = ps_pool.tile([128, HM], F32, tag="psAB", name="psB", bufs=7)
                    nc.tensor.matmul(
                        psA[:, :], uT[:, js : js + 128], lwg_t[:, :HM],
                        start=True, stop=True,
                    )
                    nc.tensor.matmul(
                        psB[:, :], uT[:, js : js + 128], lwg_t[:, HM:],
                        start=True, stop=True,
                    )
                    if evacA[ji] is nc.scalar:
                        nc.scalar.copy(osb[:, co : co + HM], psA[:, :])
                    else:
                        evacA[ji].tensor_copy(osb[:, co : co + HM], psA[:, :])
                    if evacB[ji] is nc.scalar:
                        nc.scalar.copy(osb[:, co + HM : co + M], psB[:, :])
                    else:
                        evacB[ji].tensor_copy(osb[:, co + HM : co + M], psB[:, :])
                r0 = jc0 * 128
                ab = fresh([1, 2 * BCH], F16, "pba")
                nc.gpsimd.tensor_copy(
                    ab[:, :], osb[0:1, HM - 1 : BCH * M : HM]
                )
                nc.gpsimd.dma_start(
                    out=outc[r0 : r0 + BCH * 128, :].rearrange(
                        "(j p) m -> p j m", p=128
                    ),
                    in_=osb[:, :].rearrange("p (j m) -> p j m", j=BCH),
                )
                last_ab = ab
                jc0 += BCH
    return nc


def _get_programs(A, ka):
    key = (A, ka)
    if key not in _BUILD_CACHE:
        nc1, nc2 = _build_phase1(ka), _build_phase2(A)
        _split_multiwait_waits(nc1)
        _split_multiwait_waits(nc2)
        _BUILD_CACHE[key] = (nc1, nc2)
    return _BUILD_CACHE[key]


def kernel(t, x, s, route_weights, larger_w, larger_b, elarger, tsv):
    t = int(t)
    x = np.ascontiguousarray(np.asarray(x, np.float32))
    tsv_t = np.asarray(tsv, np.float32)[t]
    allowed = np.nonzero(tsv_t != 0)[0]
    assert np.all(tsv_t[allowed] == 1.0), "tsv gate values must be 1"
    A = len(allowed)
    ka = (A + NC - 1) // NC

    nc1, nc2 = _get_programs(A, ka)

    # ---------- phase 1: priors, expert-parallel ----------
    rw = np.asarray(route_weights, np.float32)
    in1 = []
    for c in range(NC):
        xw_c = np.zeros((ka, INCH, B + ND), np.float16)
        for j in range(ka):
            g = c * ka + j
            if g < A:
                k = allowed[g]
                xw_c[j, :, :B] = x[:, k, :].T
                xw_c[j, :, B:] = rw[k].transpose(1, 0, 2).reshape(INCH, ND)
        in1.append({"xw": xw_c})
    res1 = run_bass_kernel_spmd(nc1, in1, list(range(NC)))
    LAST_RESULTS.append(res1)

    # priors_full[k, b, n, d] — stays f16 (phase-2 reads it as f16)
    priors_full = np.zeros((A, B, CAPS, D), np.float16)
    vs_full = np.zeros((B, ND), np.float32)
    for c in range(NC):
        pri = res1.results[c]["pri"]  # [ka, 2, 128, 600] f16
        vs_full += res1.results[c]["vsum"].reshape(B, ND)
        for j in range(ka):
            g = c * ka + j
            if g < A:
                priors_full[g] = pri[j].reshape(B, CAPS, D)

    # ---------- phase 2: routing + projection, pair-parallel ----------
    g_gate = 1.0 / (
        1.0 + np.exp(-(np.float32(s[0]) * np.asarray(elarger, np.float32)[t]))
    )
    lwg_f = np.asarray(larger_w, np.float32) * g_gate[:, None]  # [768, 3]
    bg = np.asarray(larger_b, np.float32) * g_gate  # [768]
    assert not np.any(bg), "nonzero larger_b not supported by this build"
    lwg_16 = np.ascontiguousarray(lwg_f.T.astype(np.float16))  # [3, 768]

    # iteration 1 on host: logit1 = g1 * <P_k, vs1>, plus its softmax pieces
    vs_v = vs_full.reshape(B, CAPS, D)
    inv_a = np.float32(1.0 / A)
    in2 = []
    for c in range(NC):
        sidx = np.arange(c * PP, (c + 1) * PP)
        nv, bv = sidx // B, sidx % B
        P2 = priors_full[:, bv, nv, :].transpose(1, 0, 2)  # [96, A, 200] f16
        vsp = vs_v[bv, nv, :]  # [96, 200] f32
        sq1 = (vsp * vsp).sum(-1) * inv_a * inv_a
        g1 = inv_a * np.sqrt(sq1) / (1.0 + sq1)
        aT1 = np.einsum("skd,sd->sk", P2.astype(np.float32), vsp)
        logit1 = (g1[:, None] * aT1).astype(np.float32)
        e2 = np.exp(logit1 - logit1.max(-1, keepdims=True))
        di2 = (1.0 / e2.sum(-1, keepdims=True)).astype(np.float32)
        in2.append(
            {
                "pri2": np.ascontiguousarray(P2.reshape(PP, A * D)),
                "lgi": logit1,
                "e2i": e2.astype(np.float32),
                "di2i": di2,
                "lwg": lwg_16,
            }
        )
    res2 = run_bass_kernel_spmd(nc2, in2, list(range(NC)))
    LAST_RESULTS.append(res2)

    out = np.concatenate(
        [res2.results[c]["outc"].astype(np.float32) for c in range(NC)], axis=0
    )
    return out.reshape(B, D, M)
